# revision 9
# baseline (speedup 1.0000x reference)
"""Fused rotary QK-projection + normalized dot-product attention softmax.

Computes softmax((q_hat @ k_hat^T) / 64) for q,k = L2-normalized rotary
projections of x, sharded over 8 NeuronCores as (batch x head-pair):
core c -> batch c//4, heads (2*(c%4), 2*(c%4)+1). No cross-core comms.

Structure: single PSUM pool of two 4-bank slots. Head-0's projection
chain (rotary, norms, normalized q/k) runs as a short prologue borrowing
the slots; the score phase then streams [128,2048] matmul->Exp->scale->
DMA row-tiles, saturating the scalar engine's exp throughput, with
head-1's chain pieces interleaved into the slot rotation between early
head-0 tiles. Norm reciprocals bounce through DRAM on the gpsimd DMA
queue; output is written bf16 (halves DMA-out bytes) and upcast on host.

Self-contained: hardcodes shapes b=2, n=2048, dim=512, h=8, d=64.
"""

import numpy as np
import ml_dtypes

B = 2
N = 2048
C = 512           # model dim (contraction for projection)
H = 8             # heads
D = 64            # head dim
HPC = 2           # heads per core
NCORES = 8
KC = C // 128     # 4 contraction chunks of 128
NJ = N // 512     # 4 chain chunks of 512
NT = N // 128     # 16 q row-tiles

_CACHE = {}


def _setup_act_tables():
    """Point walrus at an act_info.json tweaked so Ln, Exp and Square all
    resolve to natural_log_exp_and_others (one shared ACT table set -> no
    ~2.7us table reloads between activation funcs). Set order/indices are
    kept identical; only the per-func set choice is steered."""
    import os
    import json
    import tempfile
    from pathlib import Path

    if os.environ.get("BASS_ACT_ROOT_JSON_PATH"):
        return
    from neuronxcc.driver.Job import Job

    src_dir = Path(Job.getPackageDir()) / "pwp" / "pwp_bin_trainium"
    src_json = src_dir / "act_info.json"
    if not src_json.exists():
        return
    info = json.loads(src_json.read_text())
    sets = info.get("act_func_sets", [])
    names = [s.get("name") for s in sets]
    if "natural_log_exp_and_others" not in names:
        return
    for s in sets:
        if s.get("name") != "natural_log_exp_and_others":
            s.get("act", {}).pop("exp", None)
            s.get("act", {}).pop("ln", None)
            s.get("act", {}).pop("square", None)
    dst_dir = Path(tempfile.mkdtemp(prefix="pwp_act_"))
    for f in src_dir.iterdir():
        if f.name != "act_info.json":
            (dst_dir / f.name).symlink_to(f)
    (dst_dir / "act_info.json").write_text(json.dumps(info))
    os.environ["BASS_ACT_ROOT_JSON_PATH"] = str(dst_dir / "act_info.json")


def _build_nc():
    import concourse.mybir as mybir
    import concourse.tile as tile
    from concourse import bacc

    _setup_act_tables()

    dt = mybir.dt
    f32, bf16 = dt.float32, dt.bfloat16
    AF = mybir.ActivationFunctionType

    nc = bacc.Bacc(None)
    # partition-major host layouts -> contiguous per-partition DMA segments
    xT = nc.dram_tensor("xT", [128, NJ, KC, 512], bf16, kind="ExternalInput")
    wq = nc.dram_tensor("wq", [128, HPC, KC, 128], bf16, kind="ExternalInput")
    wr = nc.dram_tensor("wr", [128, HPC, KC, 128], bf16, kind="ExternalInput")
    cosr = nc.dram_tensor("cosr", [128, N], bf16, kind="ExternalInput")
    sinr = nc.dram_tensor("sinr", [128, N], bf16, kind="ExternalInput")
    maskt = nc.dram_tensor("maskt", [128, NJ, 8], bf16, kind="ExternalInput")
    out = nc.dram_tensor("out", [HPC, N, N], bf16, kind="ExternalOutput")
    # bounce rows: per-head 1/|q_n| and 1/|k_n|, broadcast-read back
    rqd = nc.dram_tensor("rqd", [HPC, N], bf16)
    rkd = nc.dram_tensor("rkd", [HPC, N], bf16)

    with tile.TileContext(nc) as tc:
        with (
            tc.tile_pool(name="singles", bufs=1) as singles,
            tc.tile_pool(name="persist", bufs=2) as persist,
            tc.tile_pool(name="chain", bufs=2) as chain_pool,
            tc.tile_pool(name="exp", bufs=3) as exp_pool,
            tc.tile_pool(name="outp", bufs=4) as out_pool,
            tc.tile_pool(name="small", bufs=4) as small,
            tc.tile_pool(name="psc", bufs=2, space="PSUM") as psc,
        ):
            wqt = singles.tile([128, HPC, KC, 128], bf16)
            nc.sync.dma_start(out=wqt[:], in_=wq[:])
            wrt = singles.tile([128, HPC, KC, 128], bf16)
            nc.sync.dma_start(out=wrt[:], in_=wr[:])
            mask4 = singles.tile([128, NJ, 8], bf16)
            nc.sync.dma_start(out=mask4[:], in_=maskt[:])
            cost = singles.tile([128, N], bf16)
            sint = singles.tile([128, N], bf16)
            xt = singles.tile([128, NJ, KC, 512], bf16)
            for j in range(NJ):
                nc.sync.dma_start(out=xt[:, j, :, :], in_=xT[:, j, :, :])
                js = slice(j * 512, (j + 1) * 512)
                nc.sync.dma_start(out=cost[:, js], in_=cosr[:, js])
                nc.sync.dma_start(out=sint[:, js], in_=sinr[:, js])

            # per-head persistent tiles: qr = [q-dims | k-dims] x n (bf16,
            # q rows normalized in place), kt = normalized k at partitions
            # 0-63.
            qr_t, kt_t = {}, {}
            t1_c, sq_c, ks_c, bq_c, bk_c, rin_c = {}, {}, {}, {}, {}, {}

            def chain_start(t):
                qr_t[t] = persist.tile([128, N], bf16, tag="qr", name=f"qr{t}")
                kt_t[t] = persist.tile([64, N], bf16, tag="kt", name=f"kt{t}")

            def chain_qk(t, j):
                # q|k projection chunk -> borrowed PSUM slot -> t1 = qk*cos
                js = slice(j * 512, (j + 1) * 512)
                qk_ps = psc.tile([128, 512], f32, tag="sc", name="qk_ps")
                for k in range(KC):
                    nc.tensor.matmul(
                        qk_ps[:], lhsT=wqt[:, t, k, :], rhs=xt[:, j, k, :],
                        start=(k == 0), stop=(k == KC - 1),
                    )
                t1 = chain_pool.tile([128, 512], bf16, tag="t1")
                nc.vector.tensor_mul(t1[:], qk_ps[:], cost[:, js])
                t1_c[t] = t1

            def chain_rot(t, j, sq_on_act):
                # rotated projection chunk -> t2 = rot*sin; qr = t1 + t2;
                # early k-half shift; squared entries for the norm matmul
                js = slice(j * 512, (j + 1) * 512)
                qr = qr_t[t]
                rot_ps = psc.tile([128, 512], f32, tag="sc", name="rot_ps")
                for k in range(KC):
                    nc.tensor.matmul(
                        rot_ps[:], lhsT=wrt[:, t, k, :], rhs=xt[:, j, k, :],
                        start=(k == 0), stop=(k == KC - 1),
                    )
                t2 = chain_pool.tile([128, 512], bf16, tag="t2")
                nc.vector.tensor_mul(t2[:], rot_ps[:], sint[:, js])
                nc.gpsimd.tensor_add(qr[:, js], t1_c[t][:], t2[:])
                ks = chain_pool.tile([64, 512], bf16, tag="ks", bufs=4)
                nc.sync.dma_start(out=ks[:], in_=qr[64:128, js])
                ks_c[(t, j)] = ks
                sq = chain_pool.tile([128, 512], bf16, tag="sq", bufs=4)
                if sq_on_act:
                    nc.scalar.activation(out=sq[:], in_=qr[:, js], func=AF.Square)
                else:
                    nc.vector.tensor_mul(sq[:], qr[:, js], qr[:, js])
                sq_c[(t, j)] = sq

            def chain_nsq(t):
                # |q|^2,|k|^2 per chunk (rows 2j,2j+1) then 1/sqrt via Ln+Exp
                nsq_ps = psc.tile([8, 512], f32, tag="sc", name="nsq_ps")
                for j in range(NJ):
                    nc.tensor.matmul(
                        nsq_ps[:], lhsT=mask4[:, j, :], rhs=sq_c[(t, j)][:],
                        start=(j == 0), stop=(j == NJ - 1),
                    )
                lnn = chain_pool.tile([8, 512], f32, tag="lnn")
                nc.scalar.activation(out=lnn[:], in_=nsq_ps[:], func=AF.Ln)
                rin = chain_pool.tile([8, 512], bf16, tag="rin")
                nc.scalar.activation(out=rin[:], in_=lnn[:], func=AF.Exp, scale=-0.5)
                rin_c[t] = rin

            def chain_fin_dma(t, jj):
                # bounce 1/|q|,1/|k| rows through DRAM and broadcast-read,
                # all on the gpsimd (SWDGE) queue: FIFO-ordered, and the
                # trigger cost stays off the scalar/sync queues
                rin = rin_c[t]
                for j in jj:
                    js = slice(j * 512, (j + 1) * 512)
                    nc.gpsimd.dma_start(out=rqd[t, js], in_=rin[2 * j:2 * j + 1, :])
                    nc.gpsimd.dma_start(out=rkd[t, js], in_=rin[2 * j + 1:2 * j + 2, :])
                    bq = chain_pool.tile([64, 512], bf16, tag="bq", bufs=4)
                    nc.gpsimd.dma_start(
                        out=bq[:], in_=rqd[t:t + 1, js].to_broadcast([64, 512])
                    )
                    bq_c[(t, j)] = bq
                    bk = chain_pool.tile([64, 512], bf16, tag="bk", bufs=4)
                    nc.gpsimd.dma_start(
                        out=bk[:], in_=rkd[t:t + 1, js].to_broadcast([64, 512])
                    )
                    bk_c[(t, j)] = bk

            def chain_fin_mul(t, jj):
                # normalize q in place (exp scale is then 1/64 const) and
                # build normalized k at partitions 0-63
                qr, kt = qr_t[t], kt_t[t]
                for j in jj:
                    js = slice(j * 512, (j + 1) * 512)
                    nc.vector.tensor_mul(qr[0:64, js], qr[0:64, js], bq_c[(t, j)][:])
                    nc.vector.tensor_mul(kt[:, js], ks_c[(t, j)][:], bk_c[(t, j)][:])

            def score_tile(t, i):
                qr, kt = qr_t[t], kt_t[t]
                isl = slice(i * 128, (i + 1) * 128)
                sc_ps = psc.tile([128, 2048], f32, tag="sc", name="sc_ps")
                for j2 in range(4):
                    nc.tensor.matmul(
                        sc_ps[:, j2 * 512:(j2 + 1) * 512],
                        lhsT=qr[0:64, isl],
                        rhs=kt[:, j2 * 512:(j2 + 1) * 512],
                        start=True, stop=True,
                    )
                et = exp_pool.tile([128, 2048], bf16, tag="et")
                sums = small.tile([128, 1], f32, tag="sums")
                nc.scalar.activation(
                    out=et[:], in_=sc_ps[:], func=AF.Exp,
                    scale=1.0 / D, accum_out=sums[:],
                )
                rs = small.tile([128, 1], f32, tag="rs")
                nc.vector.reciprocal(out=rs[:], in_=sums[:])
                ot = out_pool.tile([128, 2048], bf16, tag="ot")
                nc.vector.tensor_scalar_mul(ot[:], et[:], rs[:])
                nc.sync.dma_start(out=out[t, isl, :], in_=ot[:])

            # ---- prologue: head-0 chain ----
            chain_start(0)
            for j in range(NJ):
                chain_qk(0, j)
                chain_rot(0, j, sq_on_act=True)
            chain_nsq(0)
            chain_fin_dma(0, [0, 1])
            chain_fin_dma(0, [2, 3])
            chain_fin_mul(0, [0, 1])
            chain_fin_mul(0, [2, 3])
            chain_start(1)

            # ---- head-0 scores with head-1 chain interleaved ----
            for i in range(NT):
                score_tile(0, i)
                if 1 <= i <= 8:
                    j = (i - 1) // 2
                    if i % 2 == 1:
                        chain_qk(1, j)
                    else:
                        chain_rot(1, j, sq_on_act=False)
                elif i == 9:
                    chain_nsq(1)
                elif i == 10:
                    chain_fin_dma(1, [0, 1])
                elif i == 11:
                    chain_fin_dma(1, [2, 3])
                elif i == 12:
                    chain_fin_mul(1, [0, 1])
                elif i == 13:
                    chain_fin_mul(1, [2, 3])

            # ---- head-1 scores ----
            for i in range(NT):
                score_tile(1, i)

    nc.compile()
    return nc


def _get_nc():
    if "nc" not in _CACHE:
        _CACHE["nc"] = _build_nc()
    return _CACHE["nc"]


def _prep_inputs(x, rotary_cos, rotary_sin, W_qk):
    bf16 = ml_dtypes.bfloat16
    x = np.asarray(x, dtype=np.float32)
    cos = np.asarray(rotary_cos, dtype=np.float32)
    sin = np.asarray(rotary_sin, dtype=np.float32)
    W = np.asarray(W_qk, dtype=np.float32)

    cosr = np.concatenate([cos.T, cos.T], axis=0).astype(bf16)  # [128, N]
    sinr = np.concatenate([sin.T, sin.T], axis=0).astype(bf16)
    # nsq masks: variant j sums q-dims (partitions 0-63) into row 2j and
    # k-dims (partitions 64-127) into row 2j+1
    maskt = np.zeros((128, NJ, 8), dtype=bf16)
    for j in range(NJ):
        maskt[0:64, j, 2 * j] = 1.0
        maskt[64:128, j, 2 * j + 1] = 1.0

    # per-head weight lhsT chunks (and rotate_half-permuted variant),
    # stored partition-major: [p, head, kc, m]
    wq_h = np.empty((H, KC, 128, 128), dtype=np.float32)
    wr_h = np.empty((H, KC, 128, 128), dtype=np.float32)
    for h in range(H):
        wcat = np.concatenate(
            [W[h * D:(h + 1) * D], W[C + h * D:C + (h + 1) * D]], axis=0
        )  # [128, 512]
        wrot = np.empty_like(wcat)
        wrot[0:32] = -wcat[32:64]
        wrot[32:64] = wcat[0:32]
        wrot[64:96] = -wcat[96:128]
        wrot[96:128] = wcat[64:96]
        wq_h[h] = wcat.T.reshape(KC, 128, 128)
        wr_h[h] = wrot.T.reshape(KC, 128, 128)

    # xT partition-major chunked: [p, j, kc, nn]
    xTb = []
    for b in range(B):
        xT = x[b].T  # [C, N]
        xTb.append(np.ascontiguousarray(
            xT.reshape(KC, 128, NJ, 512).transpose(1, 2, 0, 3)
        ).astype(bf16))

    in_maps = []
    for core in range(NCORES):
        b = core // 4
        h0 = (core % 4) * HPC
        wqc = np.ascontiguousarray(
            wq_h[h0:h0 + HPC].transpose(2, 0, 1, 3)
        ).astype(bf16)  # [128, HPC, KC, 128]
        wrc = np.ascontiguousarray(
            wr_h[h0:h0 + HPC].transpose(2, 0, 1, 3)
        ).astype(bf16)
        in_maps.append({
            "xT": xTb[b],
            "wq": wqc,
            "wr": wrc,
            "cosr": cosr,
            "sinr": sinr,
            "maskt": maskt,
        })
    return in_maps


def run(x, rotary_cos, rotary_sin, W_qk, trace=False):
    from concourse.bass_utils import run_bass_kernel_spmd

    nc = _get_nc()
    in_maps = _prep_inputs(x, rotary_cos, rotary_sin, W_qk)
    res = run_bass_kernel_spmd(nc, in_maps, list(range(NCORES)), trace=trace)
    full = np.empty((B, H, N, N), dtype=np.float32)
    for core in range(NCORES):
        b = core // 4
        h0 = (core % 4) * HPC
        for t in range(HPC):
            full[b, h0 + t] = res.results[core]["out"][t].astype(np.float32)
    return full, res


def kernel(x, rotary_cos, rotary_sin, W_qk):
    full, _ = run(x, rotary_cos, rotary_sin, W_qk, trace=False)
    return full


# revision 10
# speedup vs baseline: 1.1544x; 1.1544x over previous
"""Fused rotary QK-projection + normalized dot-product attention softmax.

Computes softmax((q_hat @ k_hat^T) / 64) for q,k = L2-normalized rotary
projections of x, sharded over 8 NeuronCores as (batch x head-pair):
core c -> batch c//4, heads (2*(c%4), 2*(c%4)+1). No cross-core comms.

Structure: single PSUM pool of two 4-bank slots. Head-0's projection
chain (rotary, norms, normalized q/k) runs as a short prologue borrowing
the slots; the score phase then streams [128,2048] matmul->Exp->scale->
DMA row-tiles, saturating the scalar engine's exp throughput, with
head-1's chain pieces interleaved into the slot rotation between early
head-0 tiles. Norm reciprocals bounce through DRAM on the gpsimd DMA
queue; output is written bf16 (halves DMA-out bytes) and upcast on host.

Self-contained: hardcodes shapes b=2, n=2048, dim=512, h=8, d=64.
"""

import numpy as np
import ml_dtypes

B = 2
N = 2048
C = 512           # model dim (contraction for projection)
H = 8             # heads
D = 64            # head dim
HPC = 2           # heads per core
NCORES = 8
KC = C // 128     # 4 contraction chunks of 128
NJ = N // 512     # 4 chain chunks of 512
NT = N // 128     # 16 q row-tiles

_CACHE = {}


def _setup_act_tables():
    """Point walrus at an act_info.json tweaked so Ln, Exp and Square all
    resolve to natural_log_exp_and_others (one shared ACT table set -> no
    ~2.7us table reloads between activation funcs). Set order/indices are
    kept identical; only the per-func set choice is steered."""
    import os
    import json
    import tempfile
    from pathlib import Path

    if os.environ.get("BASS_ACT_ROOT_JSON_PATH"):
        return
    from neuronxcc.driver.Job import Job

    src_dir = Path(Job.getPackageDir()) / "pwp" / "pwp_bin_trainium"
    src_json = src_dir / "act_info.json"
    if not src_json.exists():
        return
    info = json.loads(src_json.read_text())
    sets = info.get("act_func_sets", [])
    names = [s.get("name") for s in sets]
    if "natural_log_exp_and_others" not in names:
        return
    for s in sets:
        if s.get("name") != "natural_log_exp_and_others":
            s.get("act", {}).pop("exp", None)
            s.get("act", {}).pop("ln", None)
            s.get("act", {}).pop("square", None)
    dst_dir = Path(tempfile.mkdtemp(prefix="pwp_act_"))
    for f in src_dir.iterdir():
        if f.name != "act_info.json":
            (dst_dir / f.name).symlink_to(f)
    (dst_dir / "act_info.json").write_text(json.dumps(info))
    os.environ["BASS_ACT_ROOT_JSON_PATH"] = str(dst_dir / "act_info.json")


def _build_nc():
    import concourse.mybir as mybir
    import concourse.tile as tile
    from concourse import bacc

    _setup_act_tables()

    dt = mybir.dt
    f32, bf16 = dt.float32, dt.bfloat16
    AF = mybir.ActivationFunctionType

    nc = bacc.Bacc(None)
    # partition-major host layouts -> contiguous per-partition DMA segments
    xT = nc.dram_tensor("xT", [128, NJ, KC, 512], bf16, kind="ExternalInput")
    wq = nc.dram_tensor("wq", [128, HPC, KC, 128], bf16, kind="ExternalInput")
    wr = nc.dram_tensor("wr", [128, HPC, KC, 128], bf16, kind="ExternalInput")
    cosr = nc.dram_tensor("cosr", [128, N], bf16, kind="ExternalInput")
    sinr = nc.dram_tensor("sinr", [128, N], bf16, kind="ExternalInput")
    maskt = nc.dram_tensor("maskt", [128, NJ, 8], bf16, kind="ExternalInput")
    out = nc.dram_tensor("out", [HPC, N, N], bf16, kind="ExternalOutput")
    # bounce rows: per-head 1/|q_n| and 1/|k_n|, broadcast-read back
    rqd = nc.dram_tensor("rqd", [HPC, N], bf16)
    rkd = nc.dram_tensor("rkd", [HPC, N], bf16)

    with tile.TileContext(nc) as tc:
        with (
            tc.tile_pool(name="singles", bufs=1) as singles,
            tc.tile_pool(name="persist", bufs=2) as persist,
            tc.tile_pool(name="chain", bufs=2) as chain_pool,
            tc.tile_pool(name="exp", bufs=3) as exp_pool,
            tc.tile_pool(name="outp", bufs=4) as out_pool,
            tc.tile_pool(name="small", bufs=4) as small,
            tc.tile_pool(name="psc", bufs=2, space="PSUM") as psc,
        ):
            wqt = singles.tile([128, HPC, KC, 128], bf16)
            nc.sync.dma_start(out=wqt[:], in_=wq[:])
            wrt = singles.tile([128, HPC, KC, 128], bf16)
            nc.sync.dma_start(out=wrt[:], in_=wr[:])
            mask4 = singles.tile([128, NJ, 8], bf16)
            nc.sync.dma_start(out=mask4[:], in_=maskt[:])
            cost = singles.tile([128, N], bf16)
            sint = singles.tile([128, N], bf16)
            xt = singles.tile([128, NJ, KC, 512], bf16)
            for j in range(NJ):
                nc.sync.dma_start(out=xt[:, j, :, :], in_=xT[:, j, :, :])
                js = slice(j * 512, (j + 1) * 512)
                nc.sync.dma_start(out=cost[:, js], in_=cosr[:, js])
                nc.sync.dma_start(out=sint[:, js], in_=sinr[:, js])

            # per-head persistent tiles: qr = [q-dims | k-dims] x n (bf16,
            # q rows normalized in place), kt = normalized k at partitions
            # 0-63.
            qr_t, kt_t = {}, {}
            t1_c, sq_c, ks_c, bq_c, bk_c, rin_c = {}, {}, {}, {}, {}, {}

            def chain_start(t):
                qr_t[t] = persist.tile([128, N], bf16, tag="qr", name=f"qr{t}")
                kt_t[t] = persist.tile([64, N], bf16, tag="kt", name=f"kt{t}")

            def chain_qk(t, j):
                # q|k projection chunk -> borrowed PSUM slot -> t1 = qk*cos
                js = slice(j * 512, (j + 1) * 512)
                qk_ps = psc.tile([128, 512], f32, tag="sc", name="qk_ps")
                for k in range(KC):
                    nc.tensor.matmul(
                        qk_ps[:], lhsT=wqt[:, t, k, :], rhs=xt[:, j, k, :],
                        start=(k == 0), stop=(k == KC - 1),
                    )
                t1 = chain_pool.tile([128, 512], bf16, tag="t1")
                nc.vector.tensor_mul(t1[:], qk_ps[:], cost[:, js])
                t1_c[t] = t1

            def chain_rot(t, j, sq_on_act):
                # rotated projection chunk -> t2 = rot*sin; qr = t1 + t2;
                # early k-half shift; squared entries for the norm matmul
                js = slice(j * 512, (j + 1) * 512)
                qr = qr_t[t]
                rot_ps = psc.tile([128, 512], f32, tag="sc", name="rot_ps")
                for k in range(KC):
                    nc.tensor.matmul(
                        rot_ps[:], lhsT=wrt[:, t, k, :], rhs=xt[:, j, k, :],
                        start=(k == 0), stop=(k == KC - 1),
                    )
                t2 = chain_pool.tile([128, 512], bf16, tag="t2")
                nc.vector.tensor_mul(t2[:], rot_ps[:], sint[:, js])
                nc.gpsimd.tensor_add(qr[:, js], t1_c[t][:], t2[:])
                ks = chain_pool.tile([64, 512], bf16, tag="ks", bufs=4)
                nc.sync.dma_start(out=ks[:], in_=qr[64:128, js])
                ks_c[(t, j)] = ks
                sq = chain_pool.tile([128, 512], bf16, tag="sq", bufs=4)
                if sq_on_act:
                    nc.scalar.activation(out=sq[:], in_=qr[:, js], func=AF.Square)
                else:
                    nc.vector.tensor_mul(sq[:], qr[:, js], qr[:, js])
                sq_c[(t, j)] = sq

            def chain_nsq(t):
                # |q|^2,|k|^2 per chunk (rows 2j,2j+1) then 1/sqrt via Ln+Exp
                nsq_ps = psc.tile([8, 512], f32, tag="sc", name="nsq_ps")
                for j in range(NJ):
                    nc.tensor.matmul(
                        nsq_ps[:], lhsT=mask4[:, j, :], rhs=sq_c[(t, j)][:],
                        start=(j == 0), stop=(j == NJ - 1),
                    )
                lnn = chain_pool.tile([8, 512], f32, tag="lnn")
                nc.scalar.activation(out=lnn[:], in_=nsq_ps[:], func=AF.Ln)
                rin = chain_pool.tile([8, 512], bf16, tag="rin")
                nc.scalar.activation(out=rin[:], in_=lnn[:], func=AF.Exp, scale=-0.5)
                rin_c[t] = rin

            def chain_fin_dma(t, jj):
                # bounce 1/|q|,1/|k| rows through DRAM and broadcast-read,
                # all on the gpsimd (SWDGE) queue: FIFO-ordered, and the
                # trigger cost stays off the scalar/sync queues
                rin = rin_c[t]
                for j in jj:
                    js = slice(j * 512, (j + 1) * 512)
                    nc.gpsimd.dma_start(out=rqd[t, js], in_=rin[2 * j:2 * j + 1, :])
                    nc.gpsimd.dma_start(out=rkd[t, js], in_=rin[2 * j + 1:2 * j + 2, :])
                    bq = chain_pool.tile([64, 512], bf16, tag="bq", bufs=4)
                    nc.gpsimd.dma_start(
                        out=bq[:], in_=rqd[t:t + 1, js].to_broadcast([64, 512])
                    )
                    bq_c[(t, j)] = bq
                    bk = chain_pool.tile([64, 512], bf16, tag="bk", bufs=4)
                    nc.gpsimd.dma_start(
                        out=bk[:], in_=rkd[t:t + 1, js].to_broadcast([64, 512])
                    )
                    bk_c[(t, j)] = bk

            def chain_fin_mul(t, jj):
                # normalize q in place (exp scale is then 1/64 const) and
                # build normalized k at partitions 0-63
                qr, kt = qr_t[t], kt_t[t]
                for j in jj:
                    js = slice(j * 512, (j + 1) * 512)
                    nc.vector.tensor_mul(qr[0:64, js], qr[0:64, js], bq_c[(t, j)][:])
                    nc.vector.tensor_mul(kt[:, js], ks_c[(t, j)][:], bk_c[(t, j)][:])

            def score_tile(t, i):
                qr, kt = qr_t[t], kt_t[t]
                isl = slice(i * 128, (i + 1) * 128)
                sc_ps = psc.tile([128, 2048], f32, tag="sc", name="sc_ps")
                for j2 in range(4):
                    nc.tensor.matmul(
                        sc_ps[:, j2 * 512:(j2 + 1) * 512],
                        lhsT=qr[0:64, isl],
                        rhs=kt[:, j2 * 512:(j2 + 1) * 512],
                        start=True, stop=True,
                    )
                et = exp_pool.tile([128, 2048], bf16, tag="et")
                sums = small.tile([128, 1], f32, tag="sums")
                nc.scalar.activation(
                    out=et[:], in_=sc_ps[:], func=AF.Exp,
                    scale=1.0 / D, accum_out=sums[:],
                )
                rs = small.tile([128, 1], f32, tag="rs")
                nc.vector.reciprocal(out=rs[:], in_=sums[:])
                ot = out_pool.tile([128, 2048], bf16, tag="ot")
                nc.vector.tensor_scalar_mul(ot[:], et[:], rs[:])
                nc.sync.dma_start(out=out[t, isl, :], in_=ot[:])

            # ---- prologue: head-0 chain ----
            chain_start(0)
            for j in range(NJ):
                chain_qk(0, j)
                chain_rot(0, j, sq_on_act=True)
            chain_nsq(0)
            chain_fin_dma(0, [0, 1])
            chain_fin_dma(0, [2, 3])
            chain_fin_mul(0, [0, 1])
            chain_fin_mul(0, [2, 3])
            chain_start(1)

            # ---- head-0 scores with head-1 chain interleaved ----
            # PSUM borrows come in qk+rot PAIRS after every second score
            # tile so the score tiles keep alternating between the two
            # 4-bank slots (an odd insertion would serialize fill/exp).
            for i in range(NT):
                score_tile(0, i)
                if i in (2, 4, 6, 8):
                    j = (i - 2) // 2
                    chain_qk(1, j)
                    chain_rot(1, j, sq_on_act=False)
                elif i == 9:
                    chain_nsq(1)
                elif i == 10:
                    chain_fin_dma(1, [0, 1])
                elif i == 11:
                    chain_fin_dma(1, [2, 3])
                elif i == 12:
                    chain_fin_mul(1, [0, 1])
                elif i == 13:
                    chain_fin_mul(1, [2, 3])

            # ---- head-1 scores ----
            for i in range(NT):
                score_tile(1, i)

    nc.compile()
    return nc


def _get_nc():
    if "nc" not in _CACHE:
        _CACHE["nc"] = _build_nc()
    return _CACHE["nc"]


def _prep_inputs(x, rotary_cos, rotary_sin, W_qk):
    bf16 = ml_dtypes.bfloat16
    x = np.asarray(x, dtype=np.float32)
    cos = np.asarray(rotary_cos, dtype=np.float32)
    sin = np.asarray(rotary_sin, dtype=np.float32)
    W = np.asarray(W_qk, dtype=np.float32)

    cosr = np.concatenate([cos.T, cos.T], axis=0).astype(bf16)  # [128, N]
    sinr = np.concatenate([sin.T, sin.T], axis=0).astype(bf16)
    # nsq masks: variant j sums q-dims (partitions 0-63) into row 2j and
    # k-dims (partitions 64-127) into row 2j+1
    maskt = np.zeros((128, NJ, 8), dtype=bf16)
    for j in range(NJ):
        maskt[0:64, j, 2 * j] = 1.0
        maskt[64:128, j, 2 * j + 1] = 1.0

    # per-head weight lhsT chunks (and rotate_half-permuted variant),
    # stored partition-major: [p, head, kc, m]
    wq_h = np.empty((H, KC, 128, 128), dtype=np.float32)
    wr_h = np.empty((H, KC, 128, 128), dtype=np.float32)
    for h in range(H):
        wcat = np.concatenate(
            [W[h * D:(h + 1) * D], W[C + h * D:C + (h + 1) * D]], axis=0
        )  # [128, 512]
        wrot = np.empty_like(wcat)
        wrot[0:32] = -wcat[32:64]
        wrot[32:64] = wcat[0:32]
        wrot[64:96] = -wcat[96:128]
        wrot[96:128] = wcat[64:96]
        wq_h[h] = wcat.T.reshape(KC, 128, 128)
        wr_h[h] = wrot.T.reshape(KC, 128, 128)

    # xT partition-major chunked: [p, j, kc, nn]
    xTb = []
    for b in range(B):
        xT = x[b].T  # [C, N]
        xTb.append(np.ascontiguousarray(
            xT.reshape(KC, 128, NJ, 512).transpose(1, 2, 0, 3)
        ).astype(bf16))

    in_maps = []
    for core in range(NCORES):
        b = core // 4
        h0 = (core % 4) * HPC
        wqc = np.ascontiguousarray(
            wq_h[h0:h0 + HPC].transpose(2, 0, 1, 3)
        ).astype(bf16)  # [128, HPC, KC, 128]
        wrc = np.ascontiguousarray(
            wr_h[h0:h0 + HPC].transpose(2, 0, 1, 3)
        ).astype(bf16)
        in_maps.append({
            "xT": xTb[b],
            "wq": wqc,
            "wr": wrc,
            "cosr": cosr,
            "sinr": sinr,
            "maskt": maskt,
        })
    return in_maps


def run(x, rotary_cos, rotary_sin, W_qk, trace=False):
    from concourse.bass_utils import run_bass_kernel_spmd

    nc = _get_nc()
    in_maps = _prep_inputs(x, rotary_cos, rotary_sin, W_qk)
    res = run_bass_kernel_spmd(nc, in_maps, list(range(NCORES)), trace=trace)
    full = np.empty((B, H, N, N), dtype=np.float32)
    for core in range(NCORES):
        b = core // 4
        h0 = (core % 4) * HPC
        for t in range(HPC):
            full[b, h0 + t] = res.results[core]["out"][t].astype(np.float32)
    return full, res


def kernel(x, rotary_cos, rotary_sin, W_qk):
    full, _ = run(x, rotary_cos, rotary_sin, W_qk, trace=False)
    return full


# revision 11
# speedup vs baseline: 1.3267x; 1.1492x over previous
"""Fused rotary QK-projection + normalized dot-product attention softmax.

Computes softmax((q_hat @ k_hat^T) / 64) for q,k = L2-normalized rotary
projections of x, sharded over 8 NeuronCores as (batch x head-pair):
core c -> batch c//4, heads (2*(c%4), 2*(c%4)+1). No cross-core comms.

Structure: head-0's projection chain (rotary, norms, normalized q/k)
runs as a prologue on deep chain PSUM pools; head-1's chain then
overlaps head-0's first score tiles, which run at [128,1024] exp
granularity from a 4-bank pool (phase A); the chain pools release and
the remaining tiles stream [128,2048] matmul->Exp->scale->DMA at full
width (phase B), saturating the scalar engine's exp throughput. Norm
reciprocals bounce through DRAM on the gpsimd DMA queue; 1/|q| is
multiplied into q so the exp scale is the constant 1/64. Output is
written bf16 (halves DMA-out bytes) and upcast on host.

Self-contained: hardcodes shapes b=2, n=2048, dim=512, h=8, d=64.
"""

import numpy as np
import ml_dtypes

B = 2
N = 2048
C = 512           # model dim (contraction for projection)
H = 8             # heads
D = 64            # head dim
HPC = 2           # heads per core
NCORES = 8
KC = C // 128     # 4 contraction chunks of 128
NJ = N // 512     # 4 chain chunks of 512
NT = N // 128     # 16 q row-tiles
PA = 6            # head-0 tiles run at small granularity under h1's chain

_CACHE = {}


def _setup_act_tables():
    """Point walrus at an act_info.json tweaked so Ln, Exp and Square all
    resolve to natural_log_exp_and_others (one shared ACT table set -> no
    ~2.7us table reloads between activation funcs). Set order/indices are
    kept identical; only the per-func set choice is steered."""
    import os
    import json
    import tempfile
    from pathlib import Path

    if os.environ.get("BASS_ACT_ROOT_JSON_PATH"):
        return
    from neuronxcc.driver.Job import Job

    src_dir = Path(Job.getPackageDir()) / "pwp" / "pwp_bin_trainium"
    src_json = src_dir / "act_info.json"
    if not src_json.exists():
        return
    info = json.loads(src_json.read_text())
    sets = info.get("act_func_sets", [])
    names = [s.get("name") for s in sets]
    if "natural_log_exp_and_others" not in names:
        return
    for s in sets:
        if s.get("name") != "natural_log_exp_and_others":
            s.get("act", {}).pop("exp", None)
            s.get("act", {}).pop("ln", None)
            s.get("act", {}).pop("square", None)
    dst_dir = Path(tempfile.mkdtemp(prefix="pwp_act_"))
    for f in src_dir.iterdir():
        if f.name != "act_info.json":
            (dst_dir / f.name).symlink_to(f)
    (dst_dir / "act_info.json").write_text(json.dumps(info))
    os.environ["BASS_ACT_ROOT_JSON_PATH"] = str(dst_dir / "act_info.json")


def _build_nc():
    import concourse.mybir as mybir
    import concourse.tile as tile
    from concourse import bacc

    _setup_act_tables()

    dt = mybir.dt
    f32, bf16 = dt.float32, dt.bfloat16
    AF = mybir.ActivationFunctionType

    nc = bacc.Bacc(None)
    # partition-major host layouts -> contiguous per-partition DMA segments
    xT = nc.dram_tensor("xT", [128, NJ, KC, 512], bf16, kind="ExternalInput")
    wq = nc.dram_tensor("wq", [128, HPC, KC, 128], bf16, kind="ExternalInput")
    wr = nc.dram_tensor("wr", [128, HPC, KC, 128], bf16, kind="ExternalInput")
    cosr = nc.dram_tensor("cosr", [128, N], bf16, kind="ExternalInput")
    sinr = nc.dram_tensor("sinr", [128, N], bf16, kind="ExternalInput")
    maskt = nc.dram_tensor("maskt", [128, NJ, 8], bf16, kind="ExternalInput")
    out = nc.dram_tensor("out", [HPC, N, N], bf16, kind="ExternalOutput")
    # bounce rows: per-head 1/|q_n| and 1/|k_n|, broadcast-read back
    rqd = nc.dram_tensor("rqd", [HPC, N], bf16)
    rkd = nc.dram_tensor("rkd", [HPC, N], bf16)

    with tile.TileContext(nc) as tc:
        with (
            tc.tile_pool(name="singles", bufs=1) as singles,
            tc.tile_pool(name="persist", bufs=2) as persist,
            tc.tile_pool(name="chain", bufs=2) as chain_pool,
            tc.tile_pool(name="exp", bufs=3) as exp_pool,
            tc.tile_pool(name="outp", bufs=4) as out_pool,
            tc.tile_pool(name="small", bufs=6) as small,
        ):
            wqt = singles.tile([128, HPC, KC, 128], bf16)
            nc.sync.dma_start(out=wqt[:], in_=wq[:])
            wrt = singles.tile([128, HPC, KC, 128], bf16)
            nc.sync.dma_start(out=wrt[:], in_=wr[:])
            mask4 = singles.tile([128, NJ, 8], bf16)
            nc.sync.dma_start(out=mask4[:], in_=maskt[:])
            cost = singles.tile([128, N], bf16)
            sint = singles.tile([128, N], bf16)
            xt = singles.tile([128, NJ, KC, 512], bf16)
            for j in range(NJ):
                nc.sync.dma_start(out=xt[:, j, :, :], in_=xT[:, j, :, :])
                js = slice(j * 512, (j + 1) * 512)
                nc.sync.dma_start(out=cost[:, js], in_=cosr[:, js])
                nc.sync.dma_start(out=sint[:, js], in_=sinr[:, js])

            # per-head persistent tiles: qr = [q-dims | k-dims] x n (bf16,
            # q rows normalized in place), kt = normalized k at partitions
            # 0-63.
            qr_t, kt_t = {}, {}
            t1_c, sq_c, ks_c, bq_c, bk_c, rin_c = {}, {}, {}, {}, {}, {}

            def chain_start(t):
                qr_t[t] = persist.tile([128, N], bf16, tag="qr", name=f"qr{t}")
                kt_t[t] = persist.tile([64, N], bf16, tag="kt", name=f"kt{t}")

            def chain_qk(t, j, pq_pool):
                # q|k projection chunk -> t1 = qk*cos
                js = slice(j * 512, (j + 1) * 512)
                qk_ps = pq_pool.tile([128, 512], f32, tag="pq", name="qk_ps")
                for k in range(KC):
                    nc.tensor.matmul(
                        qk_ps[:], lhsT=wqt[:, t, k, :], rhs=xt[:, j, k, :],
                        start=(k == 0), stop=(k == KC - 1),
                    )
                t1 = chain_pool.tile([128, 512], bf16, tag="t1")
                nc.vector.tensor_mul(t1[:], qk_ps[:], cost[:, js])
                t1_c[t] = t1

            def chain_rot(t, j, pr_pool, sq_on_act):
                # rotated projection chunk -> t2 = rot*sin; qr = t1 + t2;
                # early k-half shift; squared entries for the norm matmul
                js = slice(j * 512, (j + 1) * 512)
                qr = qr_t[t]
                rot_ps = pr_pool.tile([128, 512], f32, tag="pr", name="rot_ps")
                for k in range(KC):
                    nc.tensor.matmul(
                        rot_ps[:], lhsT=wrt[:, t, k, :], rhs=xt[:, j, k, :],
                        start=(k == 0), stop=(k == KC - 1),
                    )
                t2 = chain_pool.tile([128, 512], bf16, tag="t2")
                nc.vector.tensor_mul(t2[:], rot_ps[:], sint[:, js])
                nc.gpsimd.tensor_add(qr[:, js], t1_c[t][:], t2[:])
                ks = chain_pool.tile([64, 512], bf16, tag="ks", bufs=4)
                nc.sync.dma_start(out=ks[:], in_=qr[64:128, js])
                ks_c[(t, j)] = ks
                sq = chain_pool.tile([128, 512], bf16, tag="sq", bufs=4)
                if sq_on_act:
                    nc.scalar.activation(out=sq[:], in_=qr[:, js], func=AF.Square)
                else:
                    nc.vector.tensor_mul(sq[:], qr[:, js], qr[:, js])
                sq_c[(t, j)] = sq

            def chain_nsq(t, pnsq_pool):
                # |q|^2,|k|^2 per chunk (rows 2j,2j+1) then 1/sqrt via Ln+Exp
                nsq_ps = pnsq_pool.tile([8, 512], f32, tag="nsq", name="nsq_ps")
                for j in range(NJ):
                    nc.tensor.matmul(
                        nsq_ps[:], lhsT=mask4[:, j, :], rhs=sq_c[(t, j)][:],
                        start=(j == 0), stop=(j == NJ - 1),
                    )
                lnn = chain_pool.tile([8, 512], f32, tag="lnn")
                nc.scalar.activation(out=lnn[:], in_=nsq_ps[:], func=AF.Ln)
                rin = chain_pool.tile([8, 512], bf16, tag="rin")
                nc.scalar.activation(out=rin[:], in_=lnn[:], func=AF.Exp, scale=-0.5)
                rin_c[t] = rin

            def chain_fin_dma(t, jj):
                # bounce 1/|q|,1/|k| rows through DRAM and broadcast-read,
                # all on the gpsimd (SWDGE) queue: FIFO-ordered, and the
                # trigger cost stays off the scalar/sync queues
                rin = rin_c[t]
                for j in jj:
                    js = slice(j * 512, (j + 1) * 512)
                    nc.gpsimd.dma_start(out=rqd[t, js], in_=rin[2 * j:2 * j + 1, :])
                    nc.gpsimd.dma_start(out=rkd[t, js], in_=rin[2 * j + 1:2 * j + 2, :])
                    bq = chain_pool.tile([64, 512], bf16, tag="bq", bufs=4)
                    nc.gpsimd.dma_start(
                        out=bq[:], in_=rqd[t:t + 1, js].to_broadcast([64, 512])
                    )
                    bq_c[(t, j)] = bq
                    bk = chain_pool.tile([64, 512], bf16, tag="bk", bufs=4)
                    nc.gpsimd.dma_start(
                        out=bk[:], in_=rkd[t:t + 1, js].to_broadcast([64, 512])
                    )
                    bk_c[(t, j)] = bk

            def chain_fin_mul(t, jj):
                # normalize q in place (exp scale is then 1/64 const) and
                # build normalized k at partitions 0-63
                qr, kt = qr_t[t], kt_t[t]
                for j in jj:
                    js = slice(j * 512, (j + 1) * 512)
                    nc.vector.tensor_mul(qr[0:64, js], qr[0:64, js], bq_c[(t, j)][:])
                    nc.vector.tensor_mul(kt[:, js], ks_c[(t, j)][:], bk_c[(t, j)][:])

            def score_tile_small(t, i, sca_pool):
                # phase-A row-tile: two [128,1024] exp halves (pool is only
                # 4 banks while the chain pools are still alive)
                qr, kt = qr_t[t], kt_t[t]
                isl = slice(i * 128, (i + 1) * 128)
                et = exp_pool.tile([128, 2048], bf16, tag="et")
                sums2 = small.tile([128, 2], f32, tag="sums2")
                for h in range(2):
                    sc_ps = sca_pool.tile([128, 1024], f32, tag="sca", name="sca_ps")
                    for j2 in range(2):
                        jsl = slice(h * 1024 + j2 * 512, h * 1024 + (j2 + 1) * 512)
                        nc.tensor.matmul(
                            sc_ps[:, j2 * 512:(j2 + 1) * 512],
                            lhsT=qr[0:64, isl], rhs=kt[:, jsl],
                            start=True, stop=True,
                        )
                    nc.scalar.activation(
                        out=et[:, h * 1024:(h + 1) * 1024], in_=sc_ps[:],
                        func=AF.Exp, scale=1.0 / D,
                        accum_out=sums2[:, h:h + 1],
                    )
                ssum = small.tile([128, 1], f32, tag="ssum")
                nc.vector.tensor_tensor(
                    out=ssum[:], in0=sums2[:, 0:1], in1=sums2[:, 1:2],
                    op=mybir.AluOpType.add,
                )
                rs = small.tile([128, 1], f32, tag="rs")
                nc.vector.reciprocal(out=rs[:], in_=ssum[:])
                ot = out_pool.tile([128, 2048], bf16, tag="ot")
                nc.vector.tensor_scalar_mul(ot[:], et[:], rs[:])
                nc.sync.dma_start(out=out[t, isl, :], in_=ot[:])

            def score_tile(t, i, psc_pool):
                qr, kt = qr_t[t], kt_t[t]
                isl = slice(i * 128, (i + 1) * 128)
                sc_ps = psc_pool.tile([128, 2048], f32, tag="sc", name="sc_ps")
                for j2 in range(4):
                    nc.tensor.matmul(
                        sc_ps[:, j2 * 512:(j2 + 1) * 512],
                        lhsT=qr[0:64, isl],
                        rhs=kt[:, j2 * 512:(j2 + 1) * 512],
                        start=True, stop=True,
                    )
                et = exp_pool.tile([128, 2048], bf16, tag="et")
                sums = small.tile([128, 1], f32, tag="sums")
                nc.scalar.activation(
                    out=et[:], in_=sc_ps[:], func=AF.Exp,
                    scale=1.0 / D, accum_out=sums[:],
                )
                rs = small.tile([128, 1], f32, tag="rs")
                nc.vector.reciprocal(out=rs[:], in_=sums[:])
                ot = out_pool.tile([128, 2048], bf16, tag="ot")
                nc.vector.tensor_scalar_mul(ot[:], et[:], rs[:])
                nc.sync.dma_start(out=out[t, isl, :], in_=ot[:])

            # ---- prologue (head-0 chain) + phase A (h1 chain under h0
            # small-granularity score tiles); chain pools: 4 banks, phase-A
            # score pool: 4 banks ----
            with (
                tc.tile_pool(name="pq", bufs=2, space="PSUM") as pq_pool,
                tc.tile_pool(name="pr", bufs=1, space="PSUM") as pr_pool,
                tc.tile_pool(name="pnsq", bufs=1, space="PSUM") as pnsq_pool,
                tc.tile_pool(name="sca", bufs=2, space="PSUM") as sca_pool,
            ):
                chain_start(0)
                for j in range(NJ):
                    chain_qk(0, j, pq_pool)
                    chain_rot(0, j, pr_pool, sq_on_act=True)
                chain_nsq(0, pnsq_pool)
                chain_fin_dma(0, [0, 1])
                chain_fin_dma(0, [2, 3])
                chain_fin_mul(0, [0, 1])
                chain_fin_mul(0, [2, 3])
                chain_start(1)

                for j in range(NJ):
                    chain_qk(1, j, pq_pool)
                    chain_rot(1, j, pr_pool, sq_on_act=False)
                    score_tile_small(0, j, sca_pool)
                chain_nsq(1, pnsq_pool)
                score_tile_small(0, 4, sca_pool)
                chain_fin_dma(1, [0, 1])
                chain_fin_dma(1, [2, 3])
                score_tile_small(0, 5, sca_pool)
                chain_fin_mul(1, [0, 1])
                chain_fin_mul(1, [2, 3])

            # ---- phase B: full-width tiles ----
            with tc.tile_pool(name="psc", bufs=2, space="PSUM") as psc_pool:
                for i in range(PA, NT):
                    score_tile(0, i, psc_pool)
                for i in range(NT):
                    score_tile(1, i, psc_pool)

    nc.compile()
    return nc


def _get_nc():
    if "nc" not in _CACHE:
        _CACHE["nc"] = _build_nc()
    return _CACHE["nc"]


def _prep_inputs(x, rotary_cos, rotary_sin, W_qk):
    bf16 = ml_dtypes.bfloat16
    x = np.asarray(x, dtype=np.float32)
    cos = np.asarray(rotary_cos, dtype=np.float32)
    sin = np.asarray(rotary_sin, dtype=np.float32)
    W = np.asarray(W_qk, dtype=np.float32)

    cosr = np.concatenate([cos.T, cos.T], axis=0).astype(bf16)  # [128, N]
    sinr = np.concatenate([sin.T, sin.T], axis=0).astype(bf16)
    # nsq masks: variant j sums q-dims (partitions 0-63) into row 2j and
    # k-dims (partitions 64-127) into row 2j+1
    maskt = np.zeros((128, NJ, 8), dtype=bf16)
    for j in range(NJ):
        maskt[0:64, j, 2 * j] = 1.0
        maskt[64:128, j, 2 * j + 1] = 1.0

    # per-head weight lhsT chunks (and rotate_half-permuted variant),
    # stored partition-major: [p, head, kc, m]
    wq_h = np.empty((H, KC, 128, 128), dtype=np.float32)
    wr_h = np.empty((H, KC, 128, 128), dtype=np.float32)
    for h in range(H):
        wcat = np.concatenate(
            [W[h * D:(h + 1) * D], W[C + h * D:C + (h + 1) * D]], axis=0
        )  # [128, 512]
        wrot = np.empty_like(wcat)
        wrot[0:32] = -wcat[32:64]
        wrot[32:64] = wcat[0:32]
        wrot[64:96] = -wcat[96:128]
        wrot[96:128] = wcat[64:96]
        wq_h[h] = wcat.T.reshape(KC, 128, 128)
        wr_h[h] = wrot.T.reshape(KC, 128, 128)

    # xT partition-major chunked: [p, j, kc, nn]
    xTb = []
    for b in range(B):
        xT = x[b].T  # [C, N]
        xTb.append(np.ascontiguousarray(
            xT.reshape(KC, 128, NJ, 512).transpose(1, 2, 0, 3)
        ).astype(bf16))

    in_maps = []
    for core in range(NCORES):
        b = core // 4
        h0 = (core % 4) * HPC
        wqc = np.ascontiguousarray(
            wq_h[h0:h0 + HPC].transpose(2, 0, 1, 3)
        ).astype(bf16)  # [128, HPC, KC, 128]
        wrc = np.ascontiguousarray(
            wr_h[h0:h0 + HPC].transpose(2, 0, 1, 3)
        ).astype(bf16)
        in_maps.append({
            "xT": xTb[b],
            "wq": wqc,
            "wr": wrc,
            "cosr": cosr,
            "sinr": sinr,
            "maskt": maskt,
        })
    return in_maps


def run(x, rotary_cos, rotary_sin, W_qk, trace=False):
    from concourse.bass_utils import run_bass_kernel_spmd

    nc = _get_nc()
    in_maps = _prep_inputs(x, rotary_cos, rotary_sin, W_qk)
    res = run_bass_kernel_spmd(nc, in_maps, list(range(NCORES)), trace=trace)
    full = np.empty((B, H, N, N), dtype=np.float32)
    for core in range(NCORES):
        b = core // 4
        h0 = (core % 4) * HPC
        for t in range(HPC):
            full[b, h0 + t] = res.results[core]["out"][t].astype(np.float32)
    return full, res


def kernel(x, rotary_cos, rotary_sin, W_qk):
    full, _ = run(x, rotary_cos, rotary_sin, W_qk, trace=False)
    return full
